# revision 9
# baseline (speedup 1.0000x reference)
"""Windowed cross-attention (sparse_attention) on Trainium2.

Data-parallel over the batch axis across 8 NeuronCores; each core processes
16 windows (4096 tokens) of the B=128 batch. All matmuls run in float32r
(full PE rate for free-dim >= 256). Host pre-transposes x/y to feature-major
layout, folds the softmax scale into q_w, and pre-bakes the relative-position
bias per head pair so the device program is pure matmul + softmax.

v2: three-deep phase-level software pipeline. Phase n executes, interleaved
in emission order so every engine queue always has ready work:

  - attention core of super-batch n (16 kt-stages + 8 av/normalize stages)
  - q/k/v projections of super-batch n+1 (PSUM->SBUF copies on ACT/DVE)
  - output projection + bias + store of super-batch n-1
  - x/y activation loads for super-batch n+2 (single 3D-AP DMA each)

Attention per (window b2, head-pair j, k-strip kt):
  aps = I.T @ biasT (one N=512 matmul) += kT.T-slices @ qT (two N=256)
  es  = exp(aps)                               (one ACT op)
then per (b2, j): ops = [v | 1s].T @ es{kt}    (rows 0:64 raw out,
                                                rows 64:128 softmax denom)
  oT = ops[0:64] * reciprocal(ops[64:128])     (DVE only)
The ones columns of the AV lhsT are written once by Pool-engine memsets
(no per-phase DMA); the denominator rides free in the matmul M dim.
"""

import numpy as np

_TRN_REPO = "/opt/trn_rl_repo"
N_CORES = 8
B, NW, C = 128, 256, 512        # full batch, window tokens, channels
H, D = 8, 64                    # heads, head dim
WH = WW = 16
BC = B // N_CORES               # windows per core
T = BC * NW                     # tokens per core
NSB_FULL = 8                    # super-batches (2 windows each) per core
SBT = T // NSB_FULL             # tokens per super-batch


def build_module(reps=1, mm="float32r", nsb=NSB_FULL, variant="full"):
    """Build + compile the per-core Bass module (SPMD; same program all cores)."""
    import sys
    if _TRN_REPO not in sys.path:
        sys.path.insert(0, _TRN_REPO)
    from contextlib import ExitStack

    import concourse.bacc as bacc
    import concourse.tile as tile
    from concourse import mybir

    f32 = mybir.dt.float32
    mmdt = getattr(mybir.dt, mm)
    AF = mybir.ActivationFunctionType

    nc = bacc.Bacc("TRN2", debug=False, enable_asserts=False, num_devices=N_CORES)
    xT_d = nc.dram_tensor("xT", [C, T], mmdt, kind="ExternalInput")
    yT_d = nc.dram_tensor("yT", [C, T], mmdt, kind="ExternalInput")
    qw_d = nc.dram_tensor("qw", [C, C], mmdt, kind="ExternalInput")
    kvw_d = nc.dram_tensor("kvw", [C, 2 * C], mmdt, kind="ExternalInput")
    pw_d = nc.dram_tensor("pw", [C, C], mmdt, kind="ExternalInput")
    pbT_d = nc.dram_tensor("pbT", [128, 4], f32, kind="ExternalInput")
    # paired bias: bT[j, kt] = [128 k-rows, 256q(head 2j) | 256q(head 2j+1)]
    bT_d = nc.dram_tensor("bT", [H // 2, 2, 128, 2 * NW], mmdt,
                          kind="ExternalInput")
    id_d = nc.dram_tensor("ident", [128, 128], mmdt, kind="ExternalInput")
    ones_d = nc.dram_tensor("onesv", [128, H, D], mmdt, kind="ExternalInput")
    outT_d = nc.dram_tensor("outT", [C, T], f32, kind="ExternalOutput")

    xT, yT, outT = xT_d.ap(), yT_d.ap(), outT_d.ap()

    with tile.TileContext(nc) as tc, ExitStack() as ctx:
        ctx.enter_context(nc.allow_low_precision(
            reason="float32r is the matmul input format; accumulation stays fp32"))
        consts = ctx.enter_context(tc.tile_pool(name="consts", bufs=1))
        xy_pool = ctx.enter_context(tc.tile_pool(name="xy", bufs=3))
        qkv_pool = ctx.enter_context(tc.tile_pool(name="qkv", bufs=2))
        exp_pool = ctx.enter_context(tc.tile_pool(name="expp", bufs=5))
        oT_pool = ctx.enter_context(tc.tile_pool(name="oT", bufs=2))
        fin_pool = ctx.enter_context(tc.tile_pool(name="fin", bufs=2))
        small = ctx.enter_context(tc.tile_pool(name="small", bufs=2))
        pp = ctx.enter_context(tc.tile_pool(name="pp", bufs=2, space="PSUM"))
        attp = ctx.enter_context(tc.tile_pool(name="attp", bufs=4, space="PSUM"))
        op = ctx.enter_context(tc.tile_pool(name="op", bufs=2, space="PSUM"))

        # ---- constants: weights, bias, identity ----
        qw_t, kvw_t, pw_t = [], [], []
        for i in range(4):
            t = consts.tile([128, C], mmdt, name=f"qw{i}", tag=f"qw{i}")
            nc.sync.dma_start(t[:], qw_d.ap()[i * 128:(i + 1) * 128, :])
            qw_t.append(t)
        for i in range(4):
            t = consts.tile([128, 2 * C], mmdt, name=f"kvw{i}", tag=f"kvw{i}")
            nc.sync.dma_start(t[:], kvw_d.ap()[i * 128:(i + 1) * 128, :])
            kvw_t.append(t)
        for i in range(4):
            t = consts.tile([128, C], mmdt, name=f"pw{i}", tag=f"pw{i}")
            nc.sync.dma_start(t[:], pw_d.ap()[i * 128:(i + 1) * 128, :])
            pw_t.append(t)
        bT_t = [[None] * 2 for _ in range(H // 2)]
        for j in range(H // 2):
            for kt in range(2):
                t = consts.tile([128, 2 * NW], mmdt, name=f"bT{j}_{kt}",
                                tag=f"bT{j}_{kt}")
                nc.sync.dma_start(t[:], bT_d.ap()[j, kt, :, :])
                bT_t[j][kt] = t
        id_t = consts.tile([128, 128], mmdt, name="ident_t", tag="ident_t")
        nc.sync.dma_start(id_t[:], id_d.ap())
        pbT_t = consts.tile([128, 4], f32, name="pbT", tag="pbT")
        nc.sync.dma_start(pbT_t[:], pbT_d.ap())

        # per-phase live state, keyed by sb index
        live = {}

        def load_xy(n):
            if n >= nsb:
                return
            ts = n * SBT
            xt = xy_pool.tile([128, 4, SBT], mmdt, name=f"xt_{n}", tag="xt")
            for kin in range(4):
                nc.sync.dma_start(
                    xt[:, kin, :],
                    xT[kin * 128:(kin + 1) * 128, ts:ts + SBT])
            yt = xy_pool.tile([128, 4, SBT], mmdt, name=f"yt_{n}", tag="yt")
            for kin in range(4):
                nc.sync.dma_start(
                    yt[:, kin, :],
                    yT[kin * 128:(kin + 1) * 128, ts:ts + SBT])
            live.setdefault(n, {})["xt"] = xt
            live[n]["yt"] = yt

        def qkv_groups(n):
            """12 emission closures: q/k/v projection groups for sb n."""
            if n >= nsb:
                return []
            st = live[n].setdefault("qT", [None] * 4), \
                live[n].setdefault("kT", [None] * 4), \
                live[n].setdefault("vo", [None] * 4)
            qT, kT, vo = st
            xt, yt = live[n]["xt"], live[n]["yt"]
            gs = []

            def qg(m):
                ps = pp.tile([128, SBT], f32, name=f"qps_{n}_{m}", tag="pp")
                for kin in range(4):
                    nc.tensor.matmul(ps[:], qw_t[kin][:, m * 128:(m + 1) * 128],
                                     xt[:, kin, :], start=(kin == 0),
                                     stop=(kin == 3))
                qm = qkv_pool.tile([128, SBT], mmdt, name=f"qT_{n}_{m}",
                                   tag=f"q{m}")
                nc.scalar.activation(qm[:], ps[:], AF.Copy)
                qT[m] = qm

            def kg(m):
                ps = pp.tile([128, SBT], f32, name=f"kps_{n}_{m}", tag="pp")
                for kin in range(4):
                    nc.tensor.matmul(ps[:], kvw_t[kin][:, m * 128:(m + 1) * 128],
                                     yt[:, kin, :], start=(kin == 0),
                                     stop=(kin == 3))
                km = qkv_pool.tile([128, SBT], mmdt, name=f"kT_{n}_{m}",
                                   tag=f"k{m}")
                nc.scalar.activation(km[:], ps[:], AF.Copy)
                kT[m] = km

            def vg(mt):
                ps = pp.tile([128, C], f32, name=f"vps_{n}_{mt}", tag="pp")
                for kin in range(4):
                    nc.tensor.matmul(ps[:], yt[:, kin, mt * 128:(mt + 1) * 128],
                                     kvw_t[kin][:, C:2 * C],
                                     start=(kin == 0), stop=(kin == 3))
                vt = qkv_pool.tile([128, H, 2 * D], mmdt, name=f"vo_{n}_{mt}",
                                   tag=f"vo{mt}")
                nc.sync.dma_start(vt[:, :, D:2 * D], ones_d.ap())
                nc.vector.tensor_copy(vt[:, :, 0:D],
                                      ps[:].rearrange("p (h d) -> p h d", h=H))
                vo[mt] = vt

            for m in range(4):
                gs.append(lambda m=m: qg(m))
            for m in range(4):
                gs.append(lambda m=m: kg(m))
            for mt in range(4):
                gs.append(lambda mt=mt: vg(mt))
            return gs

        def oproj_groups(n):
            """4 emission closures: output projection + bias + store for sb n."""
            if n < 0:
                return []
            ts = n * SBT
            oT = live[n]["oT"]
            fin = fin_pool.tile([128, 4, SBT], f32, name=f"fin_{n}", tag="fin")
            gs = []

            def og(m):
                ps = pp.tile([128, SBT], f32, name=f"fps_{n}_{m}", tag="pp")
                for kf in range(4):
                    nc.tensor.matmul(ps[:], pw_t[kf][:, m * 128:(m + 1) * 128],
                                     oT[kf][:], start=(kf == 0), stop=(kf == 3))
                nc.scalar.activation(fin[:, m, :], ps[:], AF.Identity,
                                     bias=pbT_t[:, m:m + 1], scale=1.0)
                nc.sync.dma_start(outT[m * 128:(m + 1) * 128, ts:ts + SBT],
                                  fin[:, m, :])

            for m in range(4):
                gs.append(lambda m=m: og(m))
            return gs

        def attention_phase(n, pgs):
            """Emit attention stages of sb n interleaved with pgs closures."""
            qT, kT, vo = live[n]["qT"], live[n]["kT"], live[n]["vo"]
            oT = [oT_pool.tile([128, SBT], mmdt, name=f"oT_{n}_{m}",
                               tag=f"oT{m}") for m in range(4)]
            live[n]["oT"] = oT
            pgi = [0]

            def emit_pg():
                if pgi[0] < len(pgs):
                    pgs[pgi[0]]()
                    pgi[0] += 1

            def a_half(b2, j, kt):
                aps = attp.tile([128, SBT], f32,
                                name=f"aps_{n}_{b2}_{j}_{kt}", tag="attp")
                for hh in range(2):
                    hp = hh * 64
                    half = aps[:, hh * NW:(hh + 1) * NW]
                    nc.tensor.matmul(
                        half, id_t[:],
                        bT_t[j][kt][:, hh * NW:(hh + 1) * NW],
                        start=True, stop=False, skip_group_check=True)
                    nc.tensor.matmul(
                        half,
                        kT[j][hp:hp + 64,
                              b2 * NW + kt * 128:b2 * NW + (kt + 1) * 128],
                        qT[j][hp:hp + 64, b2 * NW:(b2 + 1) * NW],
                        start=False, stop=True, skip_group_check=True)
                e = exp_pool.tile([128, SBT], mmdt,
                                  name=f"ex_{n}_{b2}_{j}_{kt}", tag="ex")
                nc.scalar.activation(e[:], aps[:], AF.Exp)
                return e

            def b_stage(b2, j, es):
                ops_t = op.tile([128, SBT], f32, name=f"ops_{n}_{b2}_{j}",
                                tag="op")
                for hh in range(2):
                    h = 2 * j + hh
                    for kt in range(2):
                        nc.tensor.matmul(
                            ops_t[:, hh * NW:(hh + 1) * NW],
                            vo[b2 * 2 + kt][:, h, :],
                            es[kt][:, hh * NW:(hh + 1) * NW],
                            start=(kt == 0), stop=(kt == 1))
                r = small.tile([64, SBT], mmdt, name=f"r_{n}_{b2}_{j}",
                               tag="r")
                nc.vector.reciprocal(r[:], ops_t[64:128, :])
                for hh in range(2):
                    nc.vector.tensor_mul(
                        oT[j][hh * 64:(hh + 1) * 64, b2 * NW:(b2 + 1) * NW],
                        ops_t[0:64, hh * NW:(hh + 1) * NW],
                        r[:, hh * NW:(hh + 1) * NW])

            pairs = [(b2, j) for b2 in range(2) for j in range(H // 2)]
            pending = []
            for s, (b2, j) in enumerate(pairs):
                es = [a_half(b2, j, 0), a_half(b2, j, 1)]
                emit_pg()
                pending.append((b2, j, es))
                if len(pending) > 1:
                    b_stage(*pending.pop(0))
                emit_pg()
            for item in pending:
                b_stage(*item)
            while pgi[0] < len(pgs):
                emit_pg()

        def body():
            live.clear()
            # prologue: loads for sb 0/1, projections for sb 0
            load_xy(0)
            load_xy(1)
            for g in qkv_groups(0):
                g()
            for n in range(nsb):
                load_xy(n + 2)
                pgs = qkv_groups(n + 1) + oproj_groups(n - 1)
                attention_phase(n, pgs)
                if n - 2 in live:
                    del live[n - 2]
            for g in oproj_groups(nsb - 1):
                g()

        if reps == 1:
            body()
        else:
            with tc.For_i(0, reps, 1):
                body()

    nc.compile()
    return nc


def _rel_index():
    ch = np.arange(WH)
    cw = np.arange(WW)
    yy, xx = np.meshgrid(ch, cw, indexing="ij")
    coords = np.stack([yy, xx]).reshape(2, -1)           # [2, N]
    rel = coords[:, :, None] - coords[:, None, :]        # [2, N, N]
    idx = (rel[0] + WH - 1) * (2 * WW - 1) + (rel[1] + WW - 1)
    return idx                                           # [N, N] int


def make_in_maps(x, y, q_w, kv_w, proj_w, proj_b, bias_table):
    x = np.asarray(x, dtype=np.float32)
    y = np.asarray(y, dtype=np.float32)
    # fold the softmax scale into q_w
    q_w = np.ascontiguousarray(np.asarray(q_w, dtype=np.float32)) * (D ** -0.5)
    kv_w = np.ascontiguousarray(np.asarray(kv_w, dtype=np.float32))
    proj_w = np.ascontiguousarray(np.asarray(proj_w, dtype=np.float32))
    proj_b = np.asarray(proj_b, dtype=np.float32)
    bias_table = np.asarray(bias_table, dtype=np.float32)

    idx = _rel_index()
    rel_bias = bias_table[idx.reshape(-1)].reshape(NW, NW, H)   # [n1, n2, h]
    biasT = rel_bias.transpose(2, 1, 0)                         # [h, k, q]
    bT = np.empty((H // 2, 2, 128, 2 * NW), np.float32)
    for j in range(H // 2):
        for kt in range(2):
            bT[j, kt, :, 0:NW] = biasT[2 * j, kt * 128:(kt + 1) * 128, :]
            bT[j, kt, :, NW:2 * NW] = biasT[2 * j + 1, kt * 128:(kt + 1) * 128, :]
    pbT = np.ascontiguousarray(proj_b.reshape(4, 128).T)        # [128, 4]

    in_maps = []
    for c in range(N_CORES):
        xc = x[c * BC:(c + 1) * BC].reshape(T, C)
        yc = y[c * BC:(c + 1) * BC].reshape(T, C)
        in_maps.append({
            "xT": np.ascontiguousarray(xc.T),
            "yT": np.ascontiguousarray(yc.T),
            "qw": q_w, "kvw": kv_w, "pw": proj_w, "pbT": pbT, "bT": bT,
            "ident": np.eye(128, dtype=np.float32),
            "onesv": np.ones((128, H, D), np.float32),
        })
    return in_maps


_CACHE = {}


def kernel(x, y, q_w, kv_w, proj_w, proj_b, bias_table):
    import sys
    if _TRN_REPO not in sys.path:
        sys.path.insert(0, _TRN_REPO)
    from concourse.bass_utils import run_bass_kernel_spmd

    if "nc" not in _CACHE:
        _CACHE["nc"] = build_module()
    nc = _CACHE["nc"]

    in_maps = make_in_maps(x, y, q_w, kv_w, proj_w, proj_b, bias_table)
    res = run_bass_kernel_spmd(nc, in_maps, core_ids=list(range(N_CORES)))
    outs = [res.results[c]["outT"].T.reshape(BC, NW, C) for c in range(N_CORES)]
    return np.ascontiguousarray(np.concatenate(outs, axis=0), dtype=np.float32)


# revision 15
# speedup vs baseline: 1.2286x; 1.2286x over previous
"""Windowed cross-attention (sparse_attention) on Trainium2.

Data-parallel over the batch axis across 8 NeuronCores; each core processes
16 windows (4096 tokens) of the B=128 batch. All matmuls run in float32r
(full PE rate for free-dim >= 256). Host pre-transposes x/y to feature-major
layout, folds the softmax scale into q_w, and pre-bakes the relative-position
bias per head pair so the device program is pure matmul + softmax.

v2: three-deep phase-level software pipeline. Phase n executes, interleaved
in emission order so every engine queue always has ready work:

  - attention core of super-batch n (16 kt-stages + 8 av/normalize stages)
  - q/k/v projections of super-batch n+1 (PSUM->SBUF copies on ACT/DVE)
  - output projection + bias + store of super-batch n-1
  - x/y activation loads for super-batch n+2 (single 3D-AP DMA each)

Attention per (window b2, head-pair j, k-strip kt):
  aps = I.T @ biasT (one N=512 matmul) += kT.T-slices @ qT (two N=256)
  es  = exp(aps)                               (one ACT op)
then per (b2, j): ops = [v | 1s].T @ es{kt}    (rows 0:64 raw out,
                                                rows 64:128 softmax denom)
  oT = ops[0:64] * reciprocal(ops[64:128])     (DVE only)
The ones columns of the AV lhsT are written once by Pool-engine memsets
(no per-phase DMA); the denominator rides free in the matmul M dim.
"""

import numpy as np

_TRN_REPO = "/opt/trn_rl_repo"
N_CORES = 8
B, NW, C = 128, 256, 512        # full batch, window tokens, channels
H, D = 8, 64                    # heads, head dim
WH = WW = 16
BC = B // N_CORES               # windows per core
T = BC * NW                     # tokens per core
NSB_FULL = 8                    # super-batches (2 windows each) per core
SBT = T // NSB_FULL             # tokens per super-batch
MM = "bfloat16"                 # matmul input dtype (PE accumulates in fp32)


def build_module(reps=1, mm=None, nsb=NSB_FULL, variant="full"):
    """Build + compile the per-core Bass module (SPMD; same program all cores)."""
    import sys
    if _TRN_REPO not in sys.path:
        sys.path.insert(0, _TRN_REPO)
    from contextlib import ExitStack

    import concourse.bacc as bacc
    import concourse.tile as tile
    from concourse import mybir

    if mm is None:
        mm = MM

    f32 = mybir.dt.float32
    mmdt = getattr(mybir.dt, mm)
    AF = mybir.ActivationFunctionType

    nc = bacc.Bacc("TRN2", debug=False, enable_asserts=False, num_devices=N_CORES)
    xT_d = nc.dram_tensor("xT", [C, T], mmdt, kind="ExternalInput")
    yT_d = nc.dram_tensor("yT", [C, T], mmdt, kind="ExternalInput")
    qw_d = nc.dram_tensor("qw", [C, C], mmdt, kind="ExternalInput")
    kvw_d = nc.dram_tensor("kvw", [C, 2 * C], mmdt, kind="ExternalInput")
    pw_d = nc.dram_tensor("pw", [C, C], mmdt, kind="ExternalInput")
    pbT_d = nc.dram_tensor("pbT", [128, 4], f32, kind="ExternalInput")
    # paired bias: bT[j, kt] = [128 k-rows, 256q(head 2j) | 256q(head 2j+1)]
    bT_d = nc.dram_tensor("bT", [H // 2, 2, 128, 2 * NW], mmdt,
                          kind="ExternalInput")
    id_d = nc.dram_tensor("ident", [128, 128], mmdt, kind="ExternalInput")
    ones_d = nc.dram_tensor("onesv", [128, H, D], mmdt, kind="ExternalInput")
    zeros_d = nc.dram_tensor("zerosv", [64, 2 * NW], mmdt, kind="ExternalInput")
    outT_d = nc.dram_tensor("outT", [C, T], f32, kind="ExternalOutput")

    xT, yT, outT = xT_d.ap(), yT_d.ap(), outT_d.ap()

    with tile.TileContext(nc) as tc, ExitStack() as ctx:
        ctx.enter_context(nc.allow_low_precision(
            reason="float32r is the matmul input format; accumulation stays fp32"))
        consts = ctx.enter_context(tc.tile_pool(name="consts", bufs=1))
        xy_pool = ctx.enter_context(tc.tile_pool(name="xy", bufs=3))
        qkv_pool = ctx.enter_context(tc.tile_pool(name="qkv", bufs=2))
        exp_pool = ctx.enter_context(tc.tile_pool(name="expp", bufs=4))
        oT_pool = ctx.enter_context(tc.tile_pool(name="oT", bufs=2))
        fin_pool = ctx.enter_context(tc.tile_pool(name="fin", bufs=2))
        small = ctx.enter_context(tc.tile_pool(name="small", bufs=2))
        pp = ctx.enter_context(tc.tile_pool(name="pp", bufs=2, space="PSUM"))
        attp = ctx.enter_context(tc.tile_pool(name="attp", bufs=2, space="PSUM"))
        op = ctx.enter_context(tc.tile_pool(name="op", bufs=2, space="PSUM"))

        # ---- constants: weights, bias, identity ----
        qw_t, kvw_t, pw_t = [], [], []
        for i in range(4):
            t = consts.tile([128, C], mmdt, name=f"qw{i}", tag=f"qw{i}")
            nc.sync.dma_start(t[:], qw_d.ap()[i * 128:(i + 1) * 128, :])
            qw_t.append(t)
        for i in range(4):
            t = consts.tile([128, 2 * C], mmdt, name=f"kvw{i}", tag=f"kvw{i}")
            nc.sync.dma_start(t[:], kvw_d.ap()[i * 128:(i + 1) * 128, :])
            kvw_t.append(t)
        for i in range(4):
            t = consts.tile([128, C], mmdt, name=f"pw{i}", tag=f"pw{i}")
            nc.sync.dma_start(t[:], pw_d.ap()[i * 128:(i + 1) * 128, :])
            pw_t.append(t)
        bT_t = [[None] * 2 for _ in range(H // 2)]
        for j in range(H // 2):
            for kt in range(2):
                t = consts.tile([128, 2 * NW], mmdt, name=f"bT{j}_{kt}",
                                tag=f"bT{j}_{kt}")
                nc.sync.dma_start(t[:], bT_d.ap()[j, kt, :, :])
                bT_t[j][kt] = t
        id_t = consts.tile([128, 128], mmdt, name="ident_t", tag="ident_t")
        nc.sync.dma_start(id_t[:], id_d.ap())
        pbT_t = consts.tile([128, 4], f32, name="pbT", tag="pbT")
        nc.sync.dma_start(pbT_t[:], pbT_d.ap())

        # static double-buffered block-diagonal q tiles [feat, b2, hh, q]:
        # head-even q feats in rows 0:64 of hh=0 cols, head-odd in rows 64:128
        # of hh=1 cols; the off-diagonal blocks stay zero so one full-width
        # matmul against kT (both heads' k feats stacked) computes both heads'
        # scores without cross-head mixing.
        qbig = [[consts.tile([128, 2, 2, NW], mmdt, name=f"qbig{p}_{j}",
                             tag=f"qbig{p}_{j}") for j in range(H // 2)]
                for p in range(2)]
        for p in range(2):
            for j in range(H // 2):
                nc.sync.dma_start(qbig[p][j][0:64, :, 1, :],
                                  zeros_d.ap().rearrange("p (b q) -> p b q", b=2))
                nc.sync.dma_start(qbig[p][j][64:128, :, 0, :],
                                  zeros_d.ap().rearrange("p (b q) -> p b q", b=2))

        # per-phase live state, keyed by sb index
        live = {}

        def load_xy(n):
            if n >= nsb:
                return
            ts = n * SBT
            xt = xy_pool.tile([128, 4, SBT], mmdt, name=f"xt_{n}", tag="xt")
            for kin in range(4):
                nc.sync.dma_start(
                    xt[:, kin, :],
                    xT[kin * 128:(kin + 1) * 128, ts:ts + SBT])
            yt = xy_pool.tile([128, 4, SBT], mmdt, name=f"yt_{n}", tag="yt")
            for kin in range(4):
                nc.sync.dma_start(
                    yt[:, kin, :],
                    yT[kin * 128:(kin + 1) * 128, ts:ts + SBT])
            live.setdefault(n, {})["xt"] = xt
            live[n]["yt"] = yt

        def qkv_groups(n):
            """12 emission closures: q/k/v projection groups for sb n."""
            if n >= nsb:
                return []
            st = live[n].setdefault("kT", [None] * 4), \
                live[n].setdefault("vo", [None] * 4)
            kT, vo = st
            xt, yt = live[n]["xt"], live[n]["yt"]
            par = n % 2
            gs = []

            def qg(m):
                ps = pp.tile([128, SBT], f32, name=f"qps_{n}_{m}", tag="pp")
                for kin in range(4):
                    nc.tensor.matmul(ps[:], qw_t[kin][:, m * 128:(m + 1) * 128],
                                     xt[:, kin, :], start=(kin == 0),
                                     stop=(kin == 3))
                nc.scalar.activation(
                    qbig[par][m][0:64, :, 0, :],
                    ps[0:64, :].rearrange("p (b q) -> p b q", b=2), AF.Copy)
                nc.scalar.activation(
                    qbig[par][m][64:128, :, 1, :],
                    ps[64:128, :].rearrange("p (b q) -> p b q", b=2), AF.Copy)

            def kg(m):
                ps = pp.tile([128, SBT], f32, name=f"kps_{n}_{m}", tag="pp")
                for kin in range(4):
                    nc.tensor.matmul(ps[:], kvw_t[kin][:, m * 128:(m + 1) * 128],
                                     yt[:, kin, :], start=(kin == 0),
                                     stop=(kin == 3))
                km = qkv_pool.tile([128, SBT], mmdt, name=f"kT_{n}_{m}",
                                   tag=f"k{m}")
                nc.scalar.activation(km[:], ps[:], AF.Copy)
                kT[m] = km

            def vg(mt):
                ps = pp.tile([128, C], f32, name=f"vps_{n}_{mt}", tag="pp")
                for kin in range(4):
                    nc.tensor.matmul(ps[:], yt[:, kin, mt * 128:(mt + 1) * 128],
                                     kvw_t[kin][:, C:2 * C],
                                     start=(kin == 0), stop=(kin == 3))
                vt = qkv_pool.tile([128, H, 2 * D], mmdt, name=f"vo_{n}_{mt}",
                                   tag=f"vo{mt}")
                nc.sync.dma_start(vt[:, :, D:2 * D], ones_d.ap())
                nc.vector.tensor_copy(vt[:, :, 0:D],
                                      ps[:].rearrange("p (h d) -> p h d", h=H))
                vo[mt] = vt

            for m in range(4):
                gs.append(lambda m=m: qg(m))
            for m in range(4):
                gs.append(lambda m=m: kg(m))
            for mt in range(4):
                gs.append(lambda mt=mt: vg(mt))
            return gs

        def oproj_groups(n):
            """4 emission closures: output projection + bias + store for sb n."""
            if n < 0:
                return []
            ts = n * SBT
            oT = live[n]["oT"]
            fin = fin_pool.tile([128, 4, SBT], f32, name=f"fin_{n}", tag="fin")
            gs = []

            def og(m):
                ps = pp.tile([128, SBT], f32, name=f"fps_{n}_{m}", tag="pp")
                for kf in range(4):
                    nc.tensor.matmul(ps[:], pw_t[kf][:, m * 128:(m + 1) * 128],
                                     oT[kf][:], start=(kf == 0), stop=(kf == 3))
                nc.scalar.activation(fin[:, m, :], ps[:], AF.Identity,
                                     bias=pbT_t[:, m:m + 1], scale=1.0)
                nc.sync.dma_start(outT[m * 128:(m + 1) * 128, ts:ts + SBT],
                                  fin[:, m, :])

            for m in range(4):
                gs.append(lambda m=m: og(m))
            return gs

        def attention_phase(n, pgs):
            """Emit attention stages of sb n interleaved with pgs closures."""
            kT, vo = live[n]["kT"], live[n]["vo"]
            par = n % 2
            oT = [oT_pool.tile([128, SBT], mmdt, name=f"oT_{n}_{m}",
                               tag=f"oT{m}") for m in range(4)]
            live[n]["oT"] = oT
            pgi = [0]

            def emit_pg():
                if pgi[0] < len(pgs):
                    pgs[pgi[0]]()
                    pgi[0] += 1

            def a_pair(b2, j):
                aps = attp.tile([128, 2, SBT], f32,
                                name=f"aps_{n}_{b2}_{j}", tag="attp")
                for kt in range(2):
                    nc.tensor.matmul(aps[:, kt, :], id_t[:], bT_t[j][kt][:],
                                     start=True, stop=False,
                                     skip_group_check=True)
                    nc.tensor.matmul(
                        aps[:, kt, :],
                        kT[j][:, b2 * NW + kt * 128:b2 * NW + (kt + 1) * 128],
                        qbig[par][j][:, b2, :, :],
                        start=False, stop=True, skip_group_check=True)
                e = exp_pool.tile([128, 2, SBT], mmdt,
                                  name=f"ex_{n}_{b2}_{j}", tag="ex")
                nc.scalar.activation(e[:], aps[:], AF.Exp)
                return e

            def b_stage(b2, j, e):
                ops_t = op.tile([128, SBT], f32, name=f"ops_{n}_{b2}_{j}",
                                tag="op")
                for hh in range(2):
                    h = 2 * j + hh
                    for kt in range(2):
                        nc.tensor.matmul(
                            ops_t[:, hh * NW:(hh + 1) * NW],
                            vo[b2 * 2 + kt][:, h, :],
                            e[:, kt, hh * NW:(hh + 1) * NW],
                            start=(kt == 0), stop=(kt == 1))
                r = small.tile([64, SBT], mmdt, name=f"r_{n}_{b2}_{j}",
                               tag="r")
                nc.vector.reciprocal(r[:], ops_t[64:128, :])
                for hh in range(2):
                    nc.vector.tensor_mul(
                        oT[j][hh * 64:(hh + 1) * 64, b2 * NW:(b2 + 1) * NW],
                        ops_t[0:64, hh * NW:(hh + 1) * NW],
                        r[:, hh * NW:(hh + 1) * NW])

            pairs = [(b2, j) for b2 in range(2) for j in range(H // 2)]
            pending = []
            for s, (b2, j) in enumerate(pairs):
                e = a_pair(b2, j)
                emit_pg()
                pending.append((b2, j, e))
                if len(pending) > 1:
                    b_stage(*pending.pop(0))
                emit_pg()
            for item in pending:
                b_stage(*item)
            while pgi[0] < len(pgs):
                emit_pg()

        def body():
            live.clear()
            # prologue: loads for sb 0/1, projections for sb 0
            load_xy(0)
            load_xy(1)
            for g in qkv_groups(0):
                g()
            for n in range(nsb):
                load_xy(n + 2)
                pgs = qkv_groups(n + 1) + oproj_groups(n - 1)
                attention_phase(n, pgs)
                if n - 2 in live:
                    del live[n - 2]
            for g in oproj_groups(nsb - 1):
                g()

        if reps == 1:
            body()
        else:
            with tc.For_i(0, reps, 1):
                body()

    nc.compile()
    return nc


def _rel_index():
    ch = np.arange(WH)
    cw = np.arange(WW)
    yy, xx = np.meshgrid(ch, cw, indexing="ij")
    coords = np.stack([yy, xx]).reshape(2, -1)           # [2, N]
    rel = coords[:, :, None] - coords[:, None, :]        # [2, N, N]
    idx = (rel[0] + WH - 1) * (2 * WW - 1) + (rel[1] + WW - 1)
    return idx                                           # [N, N] int


def make_in_maps(x, y, q_w, kv_w, proj_w, proj_b, bias_table, mm=None):
    if mm is None:
        mm = MM
    if mm == "bfloat16":
        import ml_dtypes
        mdt = ml_dtypes.bfloat16
    else:
        mdt = np.float32

    x = np.asarray(x, dtype=np.float32)
    y = np.asarray(y, dtype=np.float32)
    # fold the softmax scale into q_w
    q_w = np.ascontiguousarray(np.asarray(q_w, dtype=np.float32)) * (D ** -0.5)
    kv_w = np.ascontiguousarray(np.asarray(kv_w, dtype=np.float32))
    proj_w = np.ascontiguousarray(np.asarray(proj_w, dtype=np.float32))
    proj_b = np.asarray(proj_b, dtype=np.float32)
    bias_table = np.asarray(bias_table, dtype=np.float32)

    idx = _rel_index()
    rel_bias = bias_table[idx.reshape(-1)].reshape(NW, NW, H)   # [n1, n2, h]
    biasT = rel_bias.transpose(2, 1, 0)                         # [h, k, q]
    bT = np.empty((H // 2, 2, 128, 2 * NW), np.float32)
    for j in range(H // 2):
        for kt in range(2):
            bT[j, kt, :, 0:NW] = biasT[2 * j, kt * 128:(kt + 1) * 128, :]
            bT[j, kt, :, NW:2 * NW] = biasT[2 * j + 1, kt * 128:(kt + 1) * 128, :]
    pbT = np.ascontiguousarray(proj_b.reshape(4, 128).T)        # [128, 4]

    in_maps = []
    for c in range(N_CORES):
        xc = x[c * BC:(c + 1) * BC].reshape(T, C)
        yc = y[c * BC:(c + 1) * BC].reshape(T, C)
        in_maps.append({
            "xT": np.ascontiguousarray(xc.T).astype(mdt),
            "yT": np.ascontiguousarray(yc.T).astype(mdt),
            "qw": q_w.astype(mdt), "kvw": kv_w.astype(mdt),
            "pw": proj_w.astype(mdt), "pbT": pbT, "bT": bT.astype(mdt),
            "ident": np.eye(128, dtype=mdt),
            "onesv": np.ones((128, H, D), mdt),
            "zerosv": np.zeros((64, 2 * NW), mdt),
        })
    return in_maps


_CACHE = {}


def kernel(x, y, q_w, kv_w, proj_w, proj_b, bias_table):
    import sys
    if _TRN_REPO not in sys.path:
        sys.path.insert(0, _TRN_REPO)
    from concourse.bass_utils import run_bass_kernel_spmd

    if "nc" not in _CACHE:
        _CACHE["nc"] = build_module()
    nc = _CACHE["nc"]

    in_maps = make_in_maps(x, y, q_w, kv_w, proj_w, proj_b, bias_table)
    res = run_bass_kernel_spmd(nc, in_maps, core_ids=list(range(N_CORES)))
    outs = [res.results[c]["outT"].T.reshape(BC, NW, C) for c in range(N_CORES)]
    return np.ascontiguousarray(np.concatenate(outs, axis=0), dtype=np.float32)
